# revision 5
# baseline (speedup 1.0000x reference)
"""Trainium2 Bass kernel for a 3-layer LIF spiking net (nn_Net_9998683865246).

Reference computation (per timestep t, 500 steps, batch 256):
    cur1 = x_t @ W1.T + b1 ; LIF1(m1)  -> s1   (128 features)
    cur2 = s1 @ W2.T + b2  ; LIF2(m2)  -> s2   (256 features)
    cur3 = s2 @ W3.T + b3  ; LIF3(m3)  -> s3   (20 features)
    out = mean_t(s3)                            [256, 20]
LIF (reset-by-subtract, reset from previous mem):
    m <- beta*m + cur - (m_prev > thr)*thr ; s = (m > thr)

Sharding: data-parallel over batch, 32 samples/core on 8 cores.

Layout: feature-on-partition, batch-on-free. All three layers' membranes are
fused into one [128, 128] state M = [m1(32 cols) | m2(64) | m3(32)], with
layer L2 lagging L1 by 2 time-blocks and L3 by 4, so each serial step is ONE
custom DVE instruction. The membrane ring is 32 deep (two block-sized halves)
so spike extraction runs ONCE PER BLOCK on gpsimd over the finished half —
no per-step cross-engine sync anywhere. The s3 16-step sums run on DVE
(tensor_reduce) lagged two blocks so their gpsimd dependency is always
already satisfied. x and weights travel as bf16 (halves HBM traffic; matmuls
run 1 cycle/row).
"""
import numpy as np
import ml_dtypes

import concourse.bass as bass
import concourse.mybir as mybir
from concourse import bacc
from concourse.tile import TileContext
from concourse.bass_utils import run_bass_kernel_spmd

# problem shape (hardcoded per harness contract)
B, T, C = 256, 500, 700
F1, F2, F3 = 128, 256, 20
NCORES = 8
NB = B // NCORES          # batch per core = 32
BLK = 16                  # timesteps per block
BLKN = BLK * NB           # matmul moving columns per block = 512
TP = 512                  # padded T
XBLK = TP // BLK          # 32 x-blocks
NBLK = XBLK + 4           # fused blocks (L3 lags by 4)
CP = 768                  # C padded to 6*128 for single-DMA x blocks
NKT = CP // 128           # 6 k-chunks of 128
FCOLS = 128               # fused state columns: 32 m1 | 64 m2 | 32 m3
RB = 2 * BLK              # membrane ring depth: two block-contiguous halves

f32 = mybir.dt.float32
bf16 = mybir.dt.bfloat16
AL = mybir.AluOpType

# ---- custom fused DVE op (registered into the concourse custom-op table) ----
# LIF_YSTEP_ANT: y' = (y*s0 + c) - [y > s1]*imm2 — one instruction advances the
# whole fused 3-layer pre-reset membrane state (y = m + BSHIFT) by one step.
from concourse.dve_spec import Spec as _Spec, Src0 as _S0, Src1 as _S1, \
    C0 as _C0, C1 as _C1, C2 as _C2
from concourse import dve_ops as _dvo


def _lif_ref(in0, in1, s0, s1, imm2):
    y = in0.astype(np.float32)
    return (y * s0 + in1) - (y > s1).astype(np.float32) * imm2


LIF_YSTEP_ANT = _dvo.DveOp(
    "LIF_YSTEP_ANT",
    _Spec(body=(_S0 * _C0 + _S1) - (_S0 > _C1) * _C2, reference=_lif_ref),
    subdim=False,
    uops_sha={"v3": "dfb1f0a941a9301a"},
)

for _op in (LIF_YSTEP_ANT,):
    if _op.name not in _dvo._SUB_OPCODE_FOR_NAME:
        _dvo.OPS.append(_op)
        _dvo._SUB_OPCODE_FOR_NAME[_op.name] = (
            _dvo._CUSTOM_DVE_ROW_BASE + len(_dvo.OPS) - 1)
        _dvo.CUSTOM_DVE_SPECS[_op.name] = _op.spec
assert max(_dvo._SUB_OPCODE_FOR_NAME.values()) < 0x20

BSHIFT = 40.0             # domain shift keeping the wrap's lower branch dead


def build_kernel(beta: float, thr: float, repeat: int = 1, skip: str = ""):
    """skip: comma-set of {c1,c2,c3,s3,extract,dma,chain} to omit (ablation)."""
    sk = set(skip.split(",")) if skip else set()
    nc = bacc.Bacc(None, target_bir_lowering=False, debug=False)

    x_in = nc.declare_dram_parameter("x", [CP, TP * NB], bf16, isOutput=False)
    w1t_in = nc.declare_dram_parameter("w1t", [CP, F1], bf16, isOutput=False)
    w2t_in = nc.declare_dram_parameter("w2t", [F1, F2], bf16, isOutput=False)
    w3t_in = nc.declare_dram_parameter("w3t", [F2, F3], bf16, isOutput=False)
    b1_in = nc.declare_dram_parameter("b1", [F1, 1], f32, isOutput=False)
    b2_in = nc.declare_dram_parameter("b2", [F2, 1], f32, isOutput=False)
    b3_in = nc.declare_dram_parameter("b3", [F3, 1], f32, isOutput=False)
    out_d = nc.declare_dram_parameter("out", [F3, NB], f32, isOutput=True)

    bound = thr + BSHIFT
    idle = (1.0 - beta) * BSHIFT

    with TileContext(nc) as tc:
        with (
            tc.tile_pool(name="wpool", bufs=1) as wpool,
            tc.tile_pool(name="xpool", bufs=2) as xpool,
            tc.tile_pool(name="cpool", bufs=3) as cpool,
            tc.tile_pool(name="spool", bufs=3) as spool,
            tc.tile_pool(name="mpool", bufs=1) as mpool,
            tc.tile_pool(name="rpool", bufs=2) as rpool,
            tc.tile_pool(name="pc1", bufs=2, space="PSUM") as pc1p,
            tc.tile_pool(name="pc2", bufs=2, space="PSUM") as pc2p,
            tc.tile_pool(name="pc3", bufs=2, space="PSUM") as pc3p,
        ):
            # ---- static weights/biases ----
            w1t = []
            for i in range(NKT):
                w = wpool.tile([128, F1], bf16, name=f"w1t{i}")
                nc.sync.dma_start(out=w[:], in_=w1t_in[i * 128:(i + 1) * 128, :])
                w1t.append(w)
            w2t = wpool.tile([F1, F2], bf16)
            nc.sync.dma_start(out=w2t[:], in_=w2t_in[:])
            w3ta = wpool.tile([128, F3], bf16)
            w3tb = wpool.tile([128, F3], bf16)
            nc.sync.dma_start(out=w3ta[:], in_=w3t_in[0:128, :])
            nc.sync.dma_start(out=w3tb[:], in_=w3t_in[128:256, :])
            b1 = wpool.tile([F1, 1], f32)
            b2a = wpool.tile([128, 1], f32)
            b2b = wpool.tile([128, 1], f32)
            b3 = wpool.tile([F3, 1], f32)
            nc.sync.dma_start(out=b1[:], in_=b1_in[:])
            nc.sync.dma_start(out=b2a[:], in_=b2_in[0:128, :])
            nc.sync.dma_start(out=b2b[:], in_=b2_in[128:256, :])
            nc.sync.dma_start(out=b3[:], in_=b3_in[:])

            o_tile = mpool.tile([F3, NB], f32)
            ssum = mpool.tile([F3, NB], f32)

            for rep in range(repeat):
                M = mpool.tile([128, RB, FCOLS], f32, name=f"M_{rep}", tag="M")
                nc.gpsimd.memset(M[:], BSHIFT)
                nc.gpsimd.memset(ssum[:], 0.0)

                xt = {}      # x tiles ring, keyed (block % 2)
                cur = {}     # cur-block ring, keyed block -> tile
                stk = {}     # S ring, keyed block -> tile

                def dma_x(j):
                    t = xpool.tile([128, NKT, BLKN], bf16, name="xblk", tag="xblk")
                    src = x_in[:].rearrange("(i p) n -> p i n", p=128)
                    nc.sync.dma_start(
                        out=t[:], in_=src[:, :, j * BLKN:(j + 1) * BLKN])
                    xt[j % 2] = t

                def prep_c1(j):
                    # cur1 for block j -> cur[j][:, :, 0:32], bias b1
                    p = pc1p.tile([F1, BLKN], f32, name="p_c1", tag="p_c1")
                    xb = xt[j % 2]
                    for i in range(NKT):
                        nc.tensor.matmul(p[:], w1t[i][:], xb[:, i, :],
                                         start=(i == 0), stop=(i == NKT - 1))
                    nc.scalar.activation(
                        cur[j][:, :, 0:32],
                        p[:].rearrange("p (k b) -> p k b", k=BLK),
                        mybir.ActivationFunctionType.Identity,
                        bias=b1[:], scale=1.0)

                def new_curblk(j):
                    t = cpool.tile([128, BLK, FCOLS], f32, name="curblk", tag="curblk")
                    cur[j] = t
                    if j < 2:
                        nc.gpsimd.memset(t[:, :, 32:128], idle)
                    elif j < 4:
                        nc.gpsimd.memset(t[:, :, 96:128], idle)
                    cur.pop(j - 3, None)

                def prep_c2(j):
                    # cur2 for block j from s1 of S[j-2] -> cur[j][:, :, 32:96]
                    if j - 2 not in stk:
                        return
                    s = stk[j - 2]
                    rhs = s[:, :, 0:32]
                    pa = pc2p.tile([128, BLKN], f32, name="p_c2a", tag="p_c2a")
                    pb = pc2p.tile([128, BLKN], f32, name="p_c2b", tag="p_c2b")
                    nc.tensor.matmul(pa[:], w2t[:, 0:128], rhs, start=True, stop=True)
                    nc.tensor.matmul(pb[:], w2t[:, 128:256], rhs, start=True, stop=True)
                    nc.scalar.activation(
                        cur[j][:, :, 32:64],
                        pa[:].rearrange("p (k b) -> p k b", k=BLK),
                        mybir.ActivationFunctionType.Identity, bias=b2a[:], scale=1.0)
                    nc.scalar.activation(
                        cur[j][:, :, 64:96],
                        pb[:].rearrange("p (k b) -> p k b", k=BLK),
                        mybir.ActivationFunctionType.Identity, bias=b2b[:], scale=1.0)

                def prep_c3(j):
                    # cur3 for block j from s2 of S[j-2] -> cur[j][0:20, :, 96:128]
                    if j - 2 not in stk:
                        return
                    s = stk[j - 2]
                    p = pc3p.tile([F3, BLKN], f32, name="p_c3", tag="p_c3")
                    nc.tensor.matmul(p[:], w3ta[:], s[:, :, 32:64], start=True, stop=False)
                    nc.tensor.matmul(p[:], w3tb[:], s[:, :, 64:96], start=False, stop=True)
                    nc.scalar.activation(
                        cur[j][0:20, :, 96:128],
                        p[:].rearrange("p (k b) -> p k b", k=BLK),
                        mybir.ActivationFunctionType.Identity, bias=b3[:], scale=1.0)

                def s3_accum(jj):
                    # ssum += sum_k s3 of S[jj] (layer-3 logical steps 16*(jj-4)+k)
                    t0 = BLK * (jj - 4)
                    kmax = min(BLK, T - t0)
                    if kmax <= 0 or jj not in stk:
                        return
                    rt = rpool.tile([F3, NB], f32, name="rt", tag="rt")
                    src = stk[jj][0:20, 0:kmax, 96:128].rearrange("p k b -> p b k")
                    nc.vector.tensor_reduce(rt[:], src, mybir.AxisListType.X, AL.add)
                    nc.gpsimd.tensor_add(ssum[:], ssum[:], rt[:])

                def extract(j):
                    # all spikes of block j in one op: S[j] = (M half > bound)
                    s_t = spool.tile([128, BLK, FCOLS], bf16, name="sblk", tag="sblk")
                    h = (j % 2) * BLK
                    nc.gpsimd.tensor_scalar(
                        s_t[:], M[:, h:h + BLK, :], bound, None, AL.is_gt)
                    stk[j] = s_t
                    stk.pop(j - 3, None)

                # ---- prologue: block 0 prep ----
                if "dma" not in sk:
                    dma_x(0)
                new_curblk(0)
                if "c1" not in sk:
                    prep_c1(0)

                for j in range(NBLK):
                    # prep cur[j+1] (runs during block j on PE/ACT/DMA)
                    if j + 1 < NBLK:
                        new_curblk(j + 1)
                        if j + 1 < XBLK:
                            if "dma" not in sk:
                                dma_x(j + 1)
                            if "c1" not in sk:
                                prep_c1(j + 1)
                        if 2 <= j + 1 and "c2" not in sk:
                            prep_c2(j + 1)
                        if 4 <= j + 1 and "c3" not in sk:
                            prep_c3(j + 1)
                    # s3 sum lagged 2 blocks: its extraction dep is long done
                    if j - 2 >= 4 and "s3" not in sk:
                        s3_accum(j - 2)

                    # serial LIF steps for block j (same-engine back-to-back)
                    h = (j % 2) * BLK
                    hp = ((j + 1) % 2) * BLK
                    if "chain" not in sk:
                        for k in range(BLK):
                            ysrc = M[:, hp + BLK - 1, :] if k == 0 else M[:, h + k - 1, :]
                            nc.vector._custom_dve(
                                LIF_YSTEP_ANT, out=M[:, h + k, :], in0=ysrc,
                                in1=cur[j][:, k, :], s0=beta, s1=bound, imm2=thr)
                    if "extract" not in sk:
                        extract(j)

                if "s3" not in sk:
                    s3_accum(NBLK - 2)
                    s3_accum(NBLK - 1)
                nc.scalar.activation(o_tile[:], ssum[:],
                                     mybir.ActivationFunctionType.Identity,
                                     bias=0.0, scale=1.0 / T)
            nc.sync.dma_start(out=out_d[:], in_=o_tile[:])
    nc.compile()
    return nc


def stage_inputs(x, W1, b1, W2, b2, W3, b3, beta, thr):
    """Build per-core input maps (host-side sharding + layout + bf16 cast)."""
    in_maps = []
    W1p = np.zeros((CP, F1), dtype=np.float32)
    W1p[:C, :] = np.ascontiguousarray(W1.T)
    W1t = W1p.astype(ml_dtypes.bfloat16)                  # [768, 128]
    W2t = np.ascontiguousarray(W2.T).astype(ml_dtypes.bfloat16)   # [128, 256]
    W3t = np.ascontiguousarray(W3.T).astype(ml_dtypes.bfloat16)   # [256, 20]
    shift = np.float32((1.0 - beta) * BSHIFT)
    b1c = np.ascontiguousarray(b1.reshape(F1, 1).astype(np.float32) + shift)
    b2c = np.ascontiguousarray(b2.reshape(F2, 1).astype(np.float32) + shift)
    b3c = np.ascontiguousarray(b3.reshape(F3, 1).astype(np.float32) + shift)
    for c in range(NCORES):
        xc = x[c * NB:(c + 1) * NB]                        # [32, 500, 700]
        xT = np.transpose(xc, (2, 1, 0))                   # [700, 500, 32]
        Xp = np.zeros((CP, TP, NB), dtype=np.float32)
        Xp[:C, :T, :] = xT
        Xc = np.ascontiguousarray(Xp.reshape(CP, TP * NB)).astype(ml_dtypes.bfloat16)
        in_maps.append({
            "x": Xc, "w1t": W1t, "w2t": W2t, "w3t": W3t,
            "b1": b1c, "b2": b2c, "b3": b3c,
        })
    return in_maps


_cache = {}
_last_result = None


def kernel(x, W1, b1, W2, b2, W3, b3,
           beta1, beta2, beta3, thr1, thr2, thr3):
    beta = float(np.clip(np.float32(beta1), 0.0, 1.0))
    thr = float(np.float32(thr1))
    assert float(beta2) == float(beta1) and float(beta3) == float(beta1)
    assert float(thr2) == float(thr1) and float(thr3) == float(thr1)

    key = (beta, thr)
    if key not in _cache:
        _cache[key] = build_kernel(beta, thr)
    nc = _cache[key]

    in_maps = stage_inputs(np.asarray(x, dtype=np.float32), np.asarray(W1), np.asarray(b1),
                           np.asarray(W2), np.asarray(b2), np.asarray(W3), np.asarray(b3),
                           beta, thr)
    res = run_bass_kernel_spmd(nc, in_maps, list(range(NCORES)))
    global _last_result
    _last_result = res
    out = np.zeros((B, F3), dtype=np.float32)
    for c in range(NCORES):
        out[c * NB:(c + 1) * NB, :] = res.results[c]["out"].T
    return out
